# revision 1
# baseline (speedup 1.0000x reference)
"""Trainium2 Bass kernel for MultiHeadAttention with relative position bias.

B=4, S=2048, D=1024, H=16, DK=64.  8 NeuronCores: core c = (batch b = c//2,
head-group g = c%2, heads g*8..g*8+8).  Host does the final 2-way partial sum
over head groups (the all-reduce after w_o).

Per-core dataflow:
  1. QT/KT = W.T @ X.T -> [512 dk, 2048 s] kept in SBUF as bf16, pair-packed
     (two heads per 128-partition tile).  V natural [2048 s, 512 dk] in SBUF
     with an appended ones column per head (col 64 of each 65-wide slot) so
     the context matmul also emits softmax row-sums.
  2. Per (head, q-half 1024), per k-tile t: scoresT tile [128 k, 1024 q] f32
     PSUM = K_h^T.T @ Q_h^T (bf16) + relative-position bias accumulated by a
     second matmul, eye(fp8) @ bias-strip, reading the strip in place from a
     per-head SBUF region slot (two slots, head-parity double buffered,
     refilled by one 0.5MB DMA per head).  exp on ScalarE (scale=1/8,
     PSUM -> SBUF bf16), 0/1 mask multiply on DVE (bf16, 2x mode); the
     masked e feeds ctx [65, 1024] PSUM += V'_h.T @ a, software-pipelined
     two k-tiles behind the scores so exp/mask latency stays off the PE
     critical path.  Row 64 = rowsum Z; drain: ScalarE row copy to
     partition 0, DVE reciprocal, Pool partition_broadcast, DVE normalize
     -> ct bf16 (head-pair-packed via DMA shift for odd heads).
  3. out[s,:] = ct.T @ WoT (partial over this core's 8 heads), host adds the
     two head-group partials per batch.

The relative-position bias regions are host-precomputed per head as fp8e4
(TRN e4m3): b8s[h, p, x] = 8 * table[x + 127 - p]; the strip for k-tile t is
the strided slice b8s[h][:, (15-t)*128 + qa : ... + 512].
"""

import numpy as np
import ml_dtypes

B, S, D = 4, 2048, 1024
H, DK = 16, 64
MAX_LEN = 2048
N_CORES = 8
HPC = 8          # heads per core
DKC = HPC * DK   # 512 dk dims per core
REG_W = S + 2048 - 128  # 3968 region width
NT = S // 128    # 16 k-tiles

_CACHE = {}


def _build_bass(debug_scratch=False, passes=1):
    import concourse.bass as bass
    import concourse.tile as tile
    import concourse.mybir as mybir
    from concourse import bacc

    f32 = mybir.dt.float32
    bf16 = mybir.dt.bfloat16
    f8e4 = mybir.dt.float8e4
    EXP = mybir.ActivationFunctionType.Exp

    nc = bacc.Bacc("TRN2", target_bir_lowering=False, debug=False,
                   num_devices=N_CORES)

    # ---- DRAM I/O (per-core) ----
    xq = nc.dram_tensor("xq", [D, S], bf16, kind="ExternalInput").ap()
    xk = nc.dram_tensor("xk", [D, S], bf16, kind="ExternalInput").ap()
    xv = nc.dram_tensor("xv", [D, S], bf16, kind="ExternalInput").ap()
    wq = nc.dram_tensor("wq", [D, DKC], bf16, kind="ExternalInput").ap()
    wk = nc.dram_tensor("wk", [D, DKC], bf16, kind="ExternalInput").ap()
    wv = nc.dram_tensor("wv", [D, DKC], bf16, kind="ExternalInput").ap()
    wo = nc.dram_tensor("wo", [HPC, DK, D], bf16, kind="ExternalInput").ap()
    mk = nc.dram_tensor("mk", [S, S], bf16, kind="ExternalInput").ap()
    b8s = nc.dram_tensor("b8s", [HPC, 128, REG_W], f8e4,
                         kind="ExternalInput").ap()
    ey = nc.dram_tensor("ey", [128, 128], f8e4, kind="ExternalInput").ap()
    out = nc.dram_tensor("out", [S, D], f32, kind="ExternalOutput").ap()

    NK = D // 128    # 8 contraction tiles for projections

    with tile.TileContext(nc) as tc:
        for _pass in range(passes):
            _sfx = '' if _pass == 0 else f'_p{_pass}'
            with tc.tile_pool(name="pers"+_sfx, bufs=1) as pers, \
                 tc.tile_pool(name="vpool"+_sfx, bufs=1) as vpool, \
                 tc.tile_pool(name="qkpool"+_sfx, bufs=1) as qkpool:

                eye8 = pers.tile([128, 128], f8e4, tag="eye8")
                nc.sync.dma_start(eye8[:], ey[:])
                mask_sb = [pers.tile([128, S], bf16, tag=f"mask{t}",
                                     name=f"mask{t}") for t in range(NT)]
                reg_sb = [pers.tile([128, REG_W], f8e4, tag=f"reg{s}",
                                    name=f"reg{s}") for s in range(2)]

                qt = [qkpool.tile([128, S], bf16, tag=f"qt{p}", name=f"qt{p}")
                      for p in range(4)]
                kt = [qkpool.tile([128, S], bf16, tag=f"kt{p}", name=f"kt{p}")
                      for p in range(4)]
                vbuf = [vpool.tile([128, HPC * 65], bf16, tag=f"vb{t}",
                                   name=f"vb{t}") for t in range(NT)]

                # ---------- phase 1: projections ----------
                with tc.tile_pool(name="pj_w"+_sfx, bufs=1) as wpool, \
                     tc.tile_pool(name="pj_x"+_sfx, bufs=2) as xpool, \
                     tc.tile_pool(name="pj_ps"+_sfx, bufs=3, space="PSUM") as pqk, \
                     tc.tile_pool(name="pj_pv"+_sfx, bufs=2, space="PSUM") as pv:
                    x_d = {"q": xq.rearrange("(k p) s -> k p s", p=128),
                           "k": xk.rearrange("(k p) s -> k p s", p=128),
                           "v": xv.rearrange("(k p) s -> k p s", p=128)}
                    w_d = {"q": wq.rearrange("(k p) n -> k p n", p=128),
                           "k": wk.rearrange("(k p) n -> k p n", p=128),
                           "v": wv.rearrange("(k p) n -> k p n", p=128)}
                    w_sb = {}

                    # ones columns of V' (col 64 of each 65-wide head slot)
                    for t in range(NT):
                        dst = vbuf[t][:].rearrange("p (h c) -> p h c", c=65)
                        nc.gpsimd.memset(dst[:, :, 64:65], 1.0)

                    def load_half(nm, sh):
                        xs = []
                        for k in range(NK):
                            if sh == 0:
                                wt = wpool.tile([128, DKC], bf16,
                                                tag=f"w{nm}{k}",
                                                name=f"w{nm}{k}")
                                nc.sync.dma_start(wt[:], w_d[nm][k])
                                w_sb[(nm, k)] = wt
                            xt = xpool.tile([128, 1024], bf16, tag=f"x{k}",
                                            name=f"x{k}")
                            nc.sync.dma_start(
                                xt[:], x_d[nm][k][:, sh * 1024:(sh + 1) * 1024])
                            xs.append(xt)
                        return xs

                    # Q and K projections, streamed in s-halves
                    for nm, dst in (("q", qt), ("k", kt)):
                        for sh in range(2):
                            xs = load_half(nm, sh)
                            if nm == "k" and sh == 1:
                                # mask tiles: DMA them after most x traffic
                                for t in range(NT):
                                    nc.sync.dma_start(
                                        mask_sb[t][:],
                                        mk[t * 128:(t + 1) * 128, :])
                            for p in range(4):
                                ps = pqk.tile([128, 1024], f32, tag="psqk")
                                # k outer / qi inner: consecutive matmuls
                                # share the stationary weight slice
                                for k in range(NK):
                                    for qi in range(2):
                                        nc.tensor.matmul(
                                            ps[:, qi * 512:(qi + 1) * 512],
                                            w_sb[(nm, k)][:,
                                                          p * 128:(p + 1) * 128],
                                            xs[k][:, qi * 512:(qi + 1) * 512],
                                            start=(k == 0), stop=(k == NK - 1))
                                if nm == "q":
                                    nc.scalar.copy(
                                        dst[p][:, sh * 1024:(sh + 1) * 1024],
                                        ps[:])
                                else:
                                    nc.vector.tensor_copy(
                                        dst[p][:, sh * 1024:(sh + 1) * 1024],
                                        ps[:])
                    # V projection
                    for sh in range(2):
                        xs = load_half("v", sh)
                        if sh == 1:
                            # bias regions for heads 0,1 into parity slots
                            for hh in (0, 1):
                                nc.sync.dma_start(reg_sb[hh][:], b8s[hh])
                        for sl in range(8):
                            st = sh * 8 + sl
                            ps = pv.tile([128, 512], f32, tag="psv")
                            for k in range(NK):
                                nc.tensor.matmul(
                                    ps[:],
                                    xs[k][:, sl * 128:(sl + 1) * 128],
                                    w_sb[("v", k)][:],
                                    start=(k == 0), stop=(k == NK - 1))
                            dst = vbuf[st][:].rearrange("p (h c) -> p h c", c=65)
                            nc.vector.tensor_copy(
                                dst[:, :, 0:64],
                                ps[:].rearrange("p (h c) -> p h c", c=64))

                # ---------- phases 2+3 ----------
                with tc.tile_pool(name="ctp"+_sfx, bufs=1) as ctpool, \
                     tc.tile_pool(name="at_e"+_sfx, bufs=4) as epool, \
                     tc.tile_pool(name="at_dr"+_sfx, bufs=2) as drpool, \
                     tc.tile_pool(name="at_wo"+_sfx, bufs=1) as wopool:
                    # ct pair-packed: head 2p in partitions 0-63, 2p+1 in
                    # 64-127, so the output projection contracts over 128
                    ct = [ctpool.tile([128, S], bf16, tag=f"ct{p}",
                                      name=f"ct{p}") for p in range(4)]
                    wo_sb = []
                    for pp in range(4):
                        wt = wopool.tile([128, D], bf16, tag=f"wo{pp}",
                                         name=f"wo{pp}")
                        nc.sync.dma_start(
                            wt[:], wo.rearrange("(p q) d k -> p (q d) k",
                                                q=2)[pp])
                        wo_sb.append(wt)

                    # ---------- phase 2: attention ----------
                    with tc.tile_pool(name="at_sc"+_sfx, bufs=2,
                                      space="PSUM") as scps, \
                         tc.tile_pool(name="at_cx"+_sfx, bufs=2,
                                      space="PSUM") as cxps:
                        def emit_drain(ctx=None, q0=None, hp=None, pi=None):
                            zr = drpool.tile([1, 1024], f32, tag="zr")
                            nc.scalar.copy(zr[:], ctx[64:65, :])
                            zri = drpool.tile([1, 1024], f32, tag="zri")
                            nc.vector.reciprocal(zri[:], zr[:])
                            rb = drpool.tile([64, 1024], f32, tag="rb")
                            nc.gpsimd.partition_broadcast(rb[:], zri[0:1, :])
                            if hp == 0:
                                nc.vector.tensor_mul(
                                    ct[pi][0:64, q0:q0 + 1024],
                                    ctx[0:64, :], rb[:])
                            else:
                                # odd head: normalize into a staging tile,
                                # DMA-shift to partitions 64-127
                                cts = drpool.tile([64, 1024], bf16,
                                                  tag="cts")
                                nc.vector.tensor_mul(
                                    cts[:], ctx[0:64, :], rb[:])
                                nc.sync.dma_start(
                                    ct[pi][64:128, q0:q0 + 1024], cts[:])

                        for h in range(HPC):
                            hp = h % 2
                            pi = h // 2
                            ktp = kt[pi]
                            qtp = qt[pi]
                            for qh in range(2):
                                q0 = qh * 1024
                                ctx = cxps.tile([65, 1024], f32, tag="ctx")
                                pend = []

                                def emit_ctx():
                                    tc_, e_ = pend.pop(0)
                                    for qi in range(2):
                                        nc.tensor.matmul(
                                            ctx[:, qi * 512:(qi + 1) * 512],
                                            vbuf[tc_][:,
                                                      h * 65:(h + 1) * 65],
                                            e_[:, qi * 512:(qi + 1) * 512],
                                            start=(tc_ == 0),
                                            stop=(tc_ == NT - 1))

                                for t in range(NT):
                                    sc = scps.tile([128, 1024], f32,
                                                   tag="sc")
                                    # both score matmuls first (shared kt
                                    # stationary), then both bias matmuls
                                    # (shared eye stationary): halves the
                                    # LDWEIGHTS traffic
                                    for qi in range(2):
                                        qa = q0 + qi * 512
                                        nc.tensor.matmul(
                                            sc[:, qi * 512:(qi + 1) * 512],
                                            ktp[hp * 64:(hp + 1) * 64,
                                                t * 128:(t + 1) * 128],
                                            qtp[hp * 64:(hp + 1) * 64,
                                                qa:qa + 512],
                                            start=True, stop=False)
                                    for qi in range(2):
                                        qa = q0 + qi * 512
                                        x0 = (NT - 1 - t) * 128 + qa
                                        nc.tensor.matmul(
                                            sc[:, qi * 512:(qi + 1) * 512],
                                            eye8[:],
                                            reg_sb[h % 2][:, x0:x0 + 512],
                                            start=False, stop=True)
                                    e = epool.tile([128, 1024], bf16,
                                                   tag="e")
                                    nc.scalar.activation(e[:], sc[:], EXP,
                                                         scale=0.125)
                                    a = epool.tile([128, 1024], bf16,
                                                   tag="a")
                                    nc.vector.tensor_mul(
                                        a[:], e[:],
                                        mask_sb[t][:, q0:q0 + 1024])
                                    pend.append((t, a))
                                    # ctx matmuls two k-tiles behind so the
                                    # exp+mask latency is off the PE
                                    # critical path
                                    if len(pend) > 2:
                                        emit_ctx()
                                while pend:
                                    emit_ctx()
                                # all reads of region slot h%2 are done
                                # for this head once qh=1 finishes: refill
                                # the slot for head h+2
                                if qh == 1 and h + 2 < HPC:
                                    nc.sync.dma_start(reg_sb[h % 2][:],
                                                      b8s[h + 2])
                                emit_drain(ctx=ctx, q0=q0, hp=hp, pi=pi)

                    # ---------- phase 3: output projection ----------
                    with tc.tile_pool(name="wo_o"+_sfx, bufs=3) as opool, \
                         tc.tile_pool(name="wo_ps"+_sfx, bufs=2,
                                      space="PSUM") as wops:
                        for st in range(NT):
                            ps = wops.tile([128, 1024], f32, tag="pso")
                            # pp outer / qi inner: reuse the ct stationary
                            for pp in range(4):
                                for qi in range(2):
                                    nc.tensor.matmul(
                                        ps[:, qi * 512:(qi + 1) * 512],
                                        ct[pp][:, st * 128:(st + 1) * 128],
                                        wo_sb[pp][:, qi * 512:(qi + 1) * 512],
                                        start=(pp == 0), stop=(pp == 3))
                            o = opool.tile([128, D], f32, tag="o")
                            if st % 2 == 0:
                                nc.scalar.copy(o[:], ps[:])
                            else:
                                nc.vector.tensor_copy(o[:], ps[:])
                            nc.sync.dma_start(
                                out[st * 128:(st + 1) * 128, :], o[:])

    nc.compile()
    return nc


def _prep_inputs(query, key, value, mask, w_q, w_k, w_v, w_o, rel_bias_table):
    """Host-side sharding prep. Returns list of per-core input dicts."""
    bf16 = ml_dtypes.bfloat16
    f8 = ml_dtypes.float8_e4m3
    tab = np.asarray(rel_bias_table, dtype=np.float32)        # [4095, 16]
    mask01 = np.asarray(mask[0, 0], dtype=np.float32)          # [S, S] (q, k)
    mkT = np.ascontiguousarray(mask01.T).astype(bf16)  # [k, q] 0/1
    eye8 = np.eye(128, dtype=np.float32).astype(f8)

    # bias strips per head-group: b8s[h, t, p, q] = 8*tab[q - 128t - p + 2047]
    b8s_g = []
    for g in range(2):
        regs = np.empty((HPC, 128, REG_W), np.float32)
        for h in range(HPC):
            col = np.ascontiguousarray(8.0 * tab[:, g * HPC + h])
            w = np.lib.stride_tricks.sliding_window_view(col, REG_W)
            regs[h] = w[::-1]
        b8s_g.append(regs.astype(f8))

    w_qT = np.ascontiguousarray(np.asarray(w_q).T).astype(bf16)
    w_kT = np.ascontiguousarray(np.asarray(w_k).T).astype(bf16)
    w_vT = np.ascontiguousarray(np.asarray(w_v).T).astype(bf16)
    w_oT = np.ascontiguousarray(np.asarray(w_o).T).astype(bf16)  # [dk_in, D]

    xq_b = [np.ascontiguousarray(np.asarray(query[b]).T).astype(bf16)
            for b in range(B)]
    xk_b = [np.ascontiguousarray(np.asarray(key[b]).T).astype(bf16)
            for b in range(B)]
    xv_b = [np.ascontiguousarray(np.asarray(value[b]).T).astype(bf16)
            for b in range(B)]

    in_maps = []
    for c in range(N_CORES):
        b, g = c // 2, c % 2
        sl = slice(g * DKC, (g + 1) * DKC)
        in_maps.append({
            "xq": xq_b[b],
            "xk": xk_b[b],
            "xv": xv_b[b],
            "wq": np.ascontiguousarray(w_qT[:, sl]),
            "wk": np.ascontiguousarray(w_kT[:, sl]),
            "wv": np.ascontiguousarray(w_vT[:, sl]),
            "wo": np.ascontiguousarray(w_oT[sl, :]).reshape(HPC, DK, D),
            "mk": mkT,
            "b8s": b8s_g[g],
            "ey": eye8,
        })
    return in_maps


def _get_exec():
    """Build (once) a persistent jitted SPMD executor for the Bass module.

    Mirrors concourse.bass2jax.run_bass_via_pjrt but caches the jitted
    callable so repeated kernel() calls skip retrace/recompile.
    """
    if "exec" in _CACHE:
        return _CACHE["exec"]

    import jax
    import jax.numpy as jnp
    from jax.sharding import Mesh, PartitionSpec
    from jax.experimental.shard_map import shard_map
    import concourse.mybir as mybir
    from concourse import bass2jax

    nc = _CACHE.get("nc")
    if nc is None:
        nc = _CACHE["nc"] = _build_bass()
    bass2jax.install_neuronx_cc_hook()

    part_name = (nc.partition_id_tensor.name
                 if nc.partition_id_tensor is not None else None)
    in_names, out_names, out_avals, zero_shapes = [], [], [], []
    for alloc in nc.m.functions[0].allocations:
        if not isinstance(alloc, mybir.MemoryLocationSet):
            continue
        name = alloc.memorylocations[0].name
        if alloc.kind == "ExternalInput":
            if name != part_name:
                in_names.append(name)
        elif alloc.kind == "ExternalOutput":
            out_names.append(name)
            shape = tuple(alloc.tensor_shape)
            dtype = mybir.dt.np(alloc.dtype)
            out_avals.append(jax.core.ShapedArray(shape, dtype))
            zero_shapes.append((shape, dtype))
    n_params = len(in_names)
    n_outs = len(out_avals)
    all_names = in_names + out_names
    if part_name is not None:
        all_names = all_names + [part_name]

    def _body(*args):
        operands = list(args)
        if part_name is not None:
            operands.append(bass2jax.partition_id_tensor())
        outs = bass2jax._bass_exec_p.bind(
            *operands,
            out_avals=tuple(out_avals),
            in_names=tuple(all_names),
            out_names=tuple(out_names),
            lowering_input_output_aliases=(),
            sim_require_finite=True,
            sim_require_nnan=True,
            nc=nc,
        )
        return tuple(outs)

    devices = jax.devices()[:N_CORES]
    mesh = Mesh(np.asarray(devices), ("core",))
    in_specs = (PartitionSpec("core"),) * (n_params + n_outs)
    out_specs = (PartitionSpec("core"),) * n_outs
    donate = tuple(range(n_params, n_params + n_outs))
    sharded = jax.jit(
        shard_map(_body, mesh=mesh, in_specs=in_specs, out_specs=out_specs,
                  check_rep=False),
        donate_argnums=donate, keep_unused=True)

    _CACHE["exec"] = (sharded, in_names, out_names, out_avals, zero_shapes)
    return _CACHE["exec"]


def _run(in_maps):
    sharded, in_names, out_names, out_avals, zero_shapes = _get_exec()
    concat_in = [np.concatenate([np.asarray(in_maps[c][nm])
                                 for c in range(N_CORES)], axis=0)
                 for nm in in_names]
    concat_zeros = [np.zeros((N_CORES * s[0], *s[1:]), d)
                    for s, d in zero_shapes]
    out_arrs = sharded(*concat_in, *concat_zeros)
    return [
        {nm: np.asarray(out_arrs[i]).reshape(N_CORES, *out_avals[i].shape)[c]
         for i, nm in enumerate(out_names)}
        for c in range(N_CORES)
    ]


def timed_run(in_maps, iters=10):
    """Steady-state timing: non-donated jit, device-resident inputs."""
    import time
    import jax
    from jax.sharding import Mesh, PartitionSpec, NamedSharding
    from jax.experimental.shard_map import shard_map
    from concourse import bass2jax

    sharded, in_names, out_names, out_avals, zero_shapes = _get_exec()
    nc = _CACHE["nc"]

    if "texec" not in _CACHE:
        import concourse.mybir as mybir
        part_name = (nc.partition_id_tensor.name
                     if nc.partition_id_tensor is not None else None)
        all_names = in_names + out_names
        if part_name is not None:
            all_names = all_names + [part_name]

        def _body(*args):
            operands = list(args)
            if part_name is not None:
                operands.append(bass2jax.partition_id_tensor())
            return tuple(bass2jax._bass_exec_p.bind(
                *operands, out_avals=tuple(out_avals), in_names=tuple(all_names),
                out_names=tuple(out_names), lowering_input_output_aliases=(),
                sim_require_finite=True, sim_require_nnan=True, nc=nc))

        devices = jax.devices()[:N_CORES]
        mesh = Mesh(np.asarray(devices), ("core",))
        n_all = len(in_names) + len(zero_shapes)
        tj = jax.jit(shard_map(_body, mesh=mesh,
                               in_specs=(PartitionSpec("core"),) * n_all,
                               out_specs=(PartitionSpec("core"),) * len(out_names),
                               check_rep=False), keep_unused=True)
        _CACHE["texec"] = (tj, mesh)
    tj, mesh = _CACHE["texec"]

    sh = NamedSharding(mesh, PartitionSpec("core"))
    concat_in = [jax.device_put(
        np.concatenate([np.asarray(in_maps[c][nm]) for c in range(N_CORES)], 0), sh)
        for nm in in_names]
    concat_zeros = [jax.device_put(np.zeros((N_CORES * s[0], *s[1:]), d), sh)
                    for s, d in zero_shapes]
    outs = tj(*concat_in, *concat_zeros)
    jax.block_until_ready(outs)
    times = []
    for _ in range(iters):
        t0 = time.perf_counter()
        outs = tj(*concat_in, *concat_zeros)
        jax.block_until_ready(outs)
        times.append(time.perf_counter() - t0)
    results = [
        {nm: np.asarray(outs[i]).reshape(N_CORES, *out_avals[i].shape)[c]
         for i, nm in enumerate(out_names)}
        for c in range(N_CORES)
    ]
    return times, results


def kernel(query, key, value, mask, w_q, b_q, w_k, b_k, w_v, b_v,
           w_o, b_o, rel_bias_table):
    in_maps = _prep_inputs(query, key, value, mask, w_q, w_k, w_v, w_o,
                           rel_bias_table)
    results = _run(in_maps)
    outs = [results[c]["out"] for c in range(N_CORES)]
    full = np.empty((B, S, D), np.float32)
    for b in range(B):
        full[b] = outs[2 * b] + outs[2 * b + 1]
    return full

